# revision 1
# baseline (speedup 1.0000x reference)
"""GNN message-passing block (edge MLP + scatter-mean + node update MLP
+ masked residual LayerNorm) on 8 Trainium2 NeuronCores.

Strategy:
  - Edges sorted by destination node; nodes split into 392 blocks of 128,
    49 blocks per core (dst-sharded => no cross-core reduction needed).
  - Per-core phase A: A = h @ W1a + (mb1 + C0), B = h @ W1b tables written
    to HBM scratch (C0 = emb[0] @ W1c folded into the bias; the edge-type
    term enters via a (C1-C0) row in the small feature matmul).
  - Edge phase, per 128-edge chunk: indirect-DMA row gathers A[src], B[dst];
    radial-basis features from host-precomputed distances; K=34 feature
    matmul; silu; mw2 matmul; silu; scatter-mean as a weighted one-hot
    matmul accumulating sums^T[h, d] in PSUM per block.
  - Node phase, per block: update MLP from sums^T, transpose, residual +
    LayerNorm + ligand mask, write output rows.

All 8 cores run an identical program (SPMD); per-block chunk counts are
padded to the max across cores at each block position.
"""

import sys

sys.path.insert(0, "/opt/trn_rl_repo")

import numpy as np
from concourse import bacc, bass, mybir
from concourse.tile import TileContext
from concourse.bass_utils import run_bass_kernel_spmd

F32 = mybir.dt.float32
I32 = mybir.dt.int32
AF = mybir.ActivationFunctionType
ALU = mybir.AluOpType

N = 50000
E = 800000
H = 128
R = 32
CUTOFF = 6.0
NCORE = 8
NB = 49                      # blocks per core
NBLK = NCORE * NB            # 392
NPAD = NBLK * 128            # 50176
GAMMA = 1.0 / max((CUTOFF / (R - 1)) ** 2, 1e-6)
LN_EPS = 1e-5

_cache = {}


def _build(kc):
    """Emit the SPMD Bacc program. kc: tuple of chunks per block position."""
    tot = sum(kc)
    nc = bacc.Bacc()

    hT = nc.declare_dram_parameter("hT", [128, NPAD], F32, isOutput=False)
    h_own = nc.declare_dram_parameter("h_own", [NB * 128, H], F32, isOutput=False)
    hTown = nc.declare_dram_parameter("hTown", [128, NB * 128], F32, isOutput=False)
    w1a = nc.declare_dram_parameter("w1a", [H, H], F32, isOutput=False)
    w1b = nc.declare_dram_parameter("w1b", [H, H], F32, isOutput=False)
    wfeat = nc.declare_dram_parameter("wfeat", [34, H], F32, isOutput=False)
    mw2 = nc.declare_dram_parameter("mw2", [H, H], F32, isOutput=False)
    utop = nc.declare_dram_parameter("utop", [H, H], F32, isOutput=False)
    ubot = nc.declare_dram_parameter("ubot", [H, H], F32, isOutput=False)
    uw2 = nc.declare_dram_parameter("uw2", [H, H], F32, isOutput=False)
    ub1 = nc.declare_dram_parameter("ub1", [H, 1], F32, isOutput=False)
    ub2 = nc.declare_dram_parameter("ub2", [H, 1], F32, isOutput=False)
    mb1c0 = nc.declare_dram_parameter("mb1c0", [128, H], F32, isOutput=False)
    mb2rep = nc.declare_dram_parameter("mb2rep", [128, H], F32, isOutput=False)
    lngrep = nc.declare_dram_parameter("lngrep", [128, H], F32, isOutput=False)
    lnbrep = nc.declare_dram_parameter("lnbrep", [128, H], F32, isOutput=False)
    centers = nc.declare_dram_parameter("centers", [128, R], F32, isOutput=False)
    iota = nc.declare_dram_parameter("iota", [128, 128], F32, isOutput=False)
    ident = nc.declare_dram_parameter("ident", [128, 128], F32, isOutput=False)
    maskf = nc.declare_dram_parameter("maskf", [128, NB], F32, isOutput=False)
    esrc = nc.declare_dram_parameter("esrc", [128, tot], I32, isOutput=False)
    edst = nc.declare_dram_parameter("edst", [128, tot], I32, isOutput=False)
    edata = nc.declare_dram_parameter("edata", [128, 4 * tot], F32, isOutput=False)
    out = nc.declare_dram_parameter("out", [NB * 128, H], F32, isOutput=True)

    A_hbm = nc.dram_tensor("A_scr", [NPAD, H], F32)
    B_hbm = nc.dram_tensor("B_scr", [NPAD, H], F32)

    with TileContext(nc) as tc:
        with (
            tc.tile_pool(name="pc", bufs=1) as pc,
            tc.tile_pool(name="pa", bufs=3) as pa,
            tc.tile_pool(name="pb", bufs=2) as pb,
            tc.tile_pool(name="pw", bufs=2) as pw,
            tc.tile_pool(name="pps", bufs=6, space="PSUM") as pps,
            tc.tile_pool(name="psums", bufs=2, space="PSUM") as psums,
        ):
            def cload(ap, shape, tag, dtype=F32):
                t = pc.tile(shape, dtype, tag=tag)
                nc.sync.dma_start(out=t[:], in_=ap[:])
                return t

            w1a_t = cload(w1a, [H, H], "w1a")
            w1b_t = cload(w1b, [H, H], "w1b")
            wfeat_t = cload(wfeat, [34, H], "wfeat")
            mw2_t = cload(mw2, [H, H], "mw2")
            utop_t = cload(utop, [H, H], "utop")
            ubot_t = cload(ubot, [H, H], "ubot")
            uw2_t = cload(uw2, [H, H], "uw2")
            ub1_t = cload(ub1, [H, 1], "ub1")
            ub2_t = cload(ub2, [H, 1], "ub2")
            mb1c0_t = cload(mb1c0, [128, H], "mb1c0")
            mb2_t = cload(mb2rep, [128, H], "mb2")
            lng_t = cload(lngrep, [128, H], "lng")
            lnb_t = cload(lnbrep, [128, H], "lnb")
            cen_t = cload(centers, [128, R], "cen")
            iota_t = cload(iota, [128, 128], "iota")
            id_t = cload(ident, [128, 128], "ident")
            mask_t = cload(maskf, [128, NB], "maskf")
            hTown_t = cload(hTown, [128, NB * 128], "hTown")

            # ---- phase A: A/B tables ----
            for cn in range(NBLK):
                sl = slice(cn * 128, (cn + 1) * 128)
                hTc = pa.tile([128, 128], F32, tag="hTc")
                nc.sync.dma_start(out=hTc[:], in_=hT[:, sl])
                pA = pps.tile([128, H], F32, tag="ps")
                nc.tensor.matmul(pA[:], hTc[:], w1a_t[:], start=True, stop=True)
                aS = pa.tile([128, H], F32, tag="aS")
                nc.vector.tensor_tensor(aS[:], pA[:], mb1c0_t[:], op=ALU.add)
                nc.sync.dma_start(out=A_hbm[sl, :], in_=aS[:])
                pB = pps.tile([128, H], F32, tag="ps")
                nc.tensor.matmul(pB[:], hTc[:], w1b_t[:], start=True, stop=True)
                bS = pa.tile([128, H], F32, tag="bS")
                nc.vector.tensor_copy(bS[:], pB[:])
                nc.sync.dma_start(out=B_hbm[sl, :], in_=bS[:])

            tc.strict_bb_all_engine_barrier()

            # ---- edge + node phases, per block ----
            q0 = 0
            for j in range(NB):
                kcj = kc[j]
                esb = pb.tile([128, kcj], I32, tag="esb")
                nc.sync.dma_start(out=esb[:], in_=esrc[:, q0 : q0 + kcj])
                edb = pb.tile([128, kcj], I32, tag="edb")
                nc.sync.dma_start(out=edb[:], in_=edst[:, q0 : q0 + kcj])
                eab = pb.tile([128, 4 * kcj], F32, tag="eab")
                nc.sync.dma_start(
                    out=eab[:], in_=edata[:, 4 * q0 : 4 * (q0 + kcj)]
                )
                sums = psums.tile([128, 128], F32, tag="sums")

                for k in range(kcj):
                    ga = pw.tile([128, H], F32, tag="ga")
                    nc.gpsimd.indirect_dma_start(
                        out=ga[:], out_offset=None, in_=A_hbm[:],
                        in_offset=bass.IndirectOffsetOnAxis(
                            ap=esb[:, k : k + 1], axis=0),
                    )
                    gb = pw.tile([128, H], F32, tag="gb")
                    nc.gpsimd.indirect_dma_start(
                        out=gb[:], out_offset=None, in_=B_hbm[:],
                        in_offset=bass.IndirectOffsetOnAxis(
                            ap=edb[:, k : k + 1], axis=0),
                    )
                    dist_ap = eab[:, 4 * k + 2 : 4 * k + 3]
                    sq = pw.tile([128, R], F32, tag="sq")
                    nc.scalar.activation(sq[:], cen_t[:], AF.Square,
                                         bias=dist_ap, scale=-1.0)
                    ft = pw.tile([128, 34], F32, tag="ft")
                    nc.scalar.activation(ft[:, 0:R], sq[:], AF.Exp, scale=-GAMMA)
                    nc.vector.tensor_copy(ft[:, R : R + 2],
                                          eab[:, 4 * k + 2 : 4 * k + 4])
                    fT = pps.tile([34, 128], F32, tag="ps")
                    nc.tensor.transpose(fT[:], ft[:], id_t[:])
                    fTs = pw.tile([34, 128], F32, tag="fTs")
                    nc.vector.tensor_copy(fTs[:], fT[:])
                    xps = pps.tile([128, H], F32, tag="ps")
                    nc.tensor.matmul(xps[:], fTs[:], wfeat_t[:],
                                     start=True, stop=True)
                    xs = pw.tile([128, H], F32, tag="xs")
                    nc.vector.tensor_tensor(xs[:], ga[:], gb[:], op=ALU.add)
                    nc.vector.tensor_tensor(xs[:], xs[:], xps[:], op=ALU.add)
                    xsl = pw.tile([128, H], F32, tag="xsl")
                    nc.scalar.activation(xsl[:], xs[:], AF.Silu)
                    xT = pps.tile([128, 128], F32, tag="ps")
                    nc.tensor.transpose(xT[:], xsl[:], id_t[:])
                    xTs = pw.tile([128, 128], F32, tag="xTs")
                    nc.vector.tensor_copy(xTs[:], xT[:])
                    yps = pps.tile([128, H], F32, tag="ps")
                    nc.tensor.matmul(yps[:], xTs[:], mw2_t[:],
                                     start=True, stop=True)
                    ms = pw.tile([128, H], F32, tag="ms")
                    nc.vector.tensor_tensor(ms[:], yps[:], mb2_t[:], op=ALU.add)
                    ms2 = pw.tile([128, H], F32, tag="ms2")
                    nc.scalar.activation(ms2[:], ms[:], AF.Silu)
                    ohw = pw.tile([128, 128], F32, tag="ohw")
                    nc.vector.tensor_scalar(
                        ohw[:], iota_t[:], eab[:, 4 * k : 4 * k + 1],
                        eab[:, 4 * k + 1 : 4 * k + 2],
                        ALU.is_equal, ALU.mult,
                    )
                    nc.tensor.matmul(sums[:], ms2[:], ohw[:],
                                     start=(k == 0), stop=(k == kcj - 1))

                # ---- node update for this block ----
                aggT = pw.tile([128, 128], F32, tag="aggT")
                nc.vector.tensor_copy(aggT[:], sums[:])
                ups = pps.tile([128, 128], F32, tag="ps")
                nc.tensor.matmul(ups[:], utop_t[:],
                                 hTown_t[:, j * 128 : (j + 1) * 128],
                                 start=True, stop=False)
                nc.tensor.matmul(ups[:], ubot_t[:], aggT[:],
                                 start=False, stop=True)
                us = pw.tile([128, 128], F32, tag="us")
                nc.scalar.activation(us[:], ups[:], AF.Silu, bias=ub1_t[:, 0:1])
                uds = pps.tile([128, 128], F32, tag="ps")
                nc.tensor.matmul(uds[:], uw2_t[:], us[:], start=True, stop=True)
                udb = pw.tile([128, 128], F32, tag="udb")
                nc.vector.tensor_scalar(udb[:], uds[:], ub2_t[:, 0:1], None,
                                        ALU.add)
                updp = pps.tile([128, 128], F32, tag="ps")
                nc.tensor.transpose(updp[:], udb[:], id_t[:])
                hb = pb.tile([128, 128], F32, tag="hb")
                nc.sync.dma_start(out=hb[:],
                                  in_=h_own[j * 128 : (j + 1) * 128, :])
                z = pw.tile([128, H], F32, tag="z")
                nc.vector.tensor_tensor(z[:], updp[:], hb[:], op=ALU.add)
                mu = pw.tile([128, 1], F32, tag="mu")
                nc.vector.tensor_reduce(mu[:], z[:], mybir.AxisListType.X,
                                        ALU.add)
                nc.vector.tensor_scalar(mu[:], mu[:], 1.0 / H, None, ALU.mult)
                zc = pw.tile([128, H], F32, tag="zc")
                nc.vector.tensor_scalar(zc[:], z[:], mu[:, 0:1], None,
                                        ALU.subtract)
                sqd = pw.tile([128, H], F32, tag="sqd")
                ss = pw.tile([128, 1], F32, tag="ss")
                nc.scalar.activation(sqd[:], zc[:], AF.Square, accum_out=ss[:])
                ra = pw.tile([128, 1], F32, tag="ra")
                nc.vector.tensor_scalar(ra[:], ss[:], 1.0 / H, LN_EPS,
                                        ALU.mult, ALU.add)
                sd = pw.tile([128, 1], F32, tag="sd")
                nc.scalar.activation(sd[:], ra[:], AF.Sqrt)
                rs = pw.tile([128, 1], F32, tag="rs")
                nc.vector.reciprocal(rs[:], sd[:])
                nm = pw.tile([128, H], F32, tag="nm")
                nc.vector.tensor_scalar(nm[:], zc[:], rs[:, 0:1], None, ALU.mult)
                nc.vector.tensor_tensor(nm[:], nm[:], lng_t[:], op=ALU.mult)
                nc.vector.tensor_tensor(nm[:], nm[:], lnb_t[:], op=ALU.add)
                d1 = pw.tile([128, H], F32, tag="d1")
                nc.vector.tensor_tensor(d1[:], nm[:], hb[:], op=ALU.subtract)
                nc.vector.tensor_scalar(d1[:], d1[:], mask_t[:, j : j + 1],
                                        None, ALU.mult)
                nc.vector.tensor_tensor(d1[:], d1[:], hb[:], op=ALU.add)
                nc.sync.dma_start(out=out[j * 128 : (j + 1) * 128, :], in_=d1[:])
                q0 += kcj

    nc.compile()
    return nc


def _prep(h, pos, edge_index, edge_type, node_type,
          emb, mw1, mb1, mw2, mb2, uw1, ub1, uw2, ub2, ln_g, ln_b):
    h = np.asarray(h, np.float32)
    pos = np.asarray(pos, np.float32)
    src = np.asarray(edge_index[0], np.int64)
    dst = np.asarray(edge_index[1], np.int64)
    et = np.asarray(edge_type, np.int64)
    ntype = np.asarray(node_type)
    mw1 = np.asarray(mw1, np.float32)
    emb = np.asarray(emb, np.float32)

    blk = dst >> 7
    order = np.lexsort((src, blk))
    src_s = src[order]
    dst_s = dst[order]
    blk_s = blk[order]
    cnt = np.bincount(dst, minlength=N).astype(np.float32)
    w_s = (1.0 / np.maximum(cnt, 1.0))[dst_s].astype(np.float32)
    rel = pos[src_s] - pos[dst_s]
    dist_s = np.sqrt((rel * rel).sum(axis=1)).astype(np.float32)
    dl_s = (dst_s & 127).astype(np.float32)
    et_s = et[order].astype(np.float32)

    bc = np.bincount(blk_s, minlength=NBLK)
    bstart = np.zeros(NBLK + 1, np.int64)
    np.cumsum(bc, out=bstart[1:])
    cnts = bc.reshape(NCORE, NB)
    kc = np.maximum(1, (cnts + 127) // 128).max(axis=0)
    tot = int(kc.sum())

    per_core = []
    for c in range(NCORE):
        fsrc = np.zeros(tot * 128, np.int32)
        fdst = np.zeros(tot * 128, np.int32)
        fdat = np.zeros((tot * 128, 4), np.float32)
        fdat[:, 2] = 1.0  # dummy dist, benign
        base = 0
        for j in range(NB):
            g = c * NB + j
            s0, s1 = bstart[g], bstart[g + 1]
            n = s1 - s0
            fsrc[base : base + n] = src_s[s0:s1]
            fdst[base : base + n] = dst_s[s0:s1]
            fdat[base : base + n, 0] = dl_s[s0:s1]
            fdat[base : base + n, 1] = w_s[s0:s1]
            fdat[base : base + n, 2] = dist_s[s0:s1]
            fdat[base : base + n, 3] = et_s[s0:s1]
            base += int(kc[j]) * 128
        esrc2 = fsrc.reshape(tot, 128).T.copy()
        edst2 = fdst.reshape(tot, 128).T.copy()
        edata2 = np.ascontiguousarray(
            fdat.reshape(tot, 128, 4).transpose(1, 0, 2).reshape(128, 4 * tot))
        per_core.append((esrc2, edst2, edata2))

    hT = np.zeros((128, NPAD), np.float32)
    hT[:, :N] = h.T
    h_pad = np.zeros((NPAD, H), np.float32)
    h_pad[:N] = h
    maskp = np.zeros(NPAD, np.float32)
    maskp[:N] = (np.asarray(ntype) == 0).astype(np.float32)

    W1a = np.ascontiguousarray(mw1[0:128])
    W1b = np.ascontiguousarray(mw1[128:256])
    W1c = mw1[256:384]
    W1d = mw1[384:416]
    w1e = mw1[416:417]
    C = emb @ W1c  # [2, H]
    wfeat = np.ascontiguousarray(
        np.vstack([W1d, w1e, (C[1] - C[0])[None, :]]).astype(np.float32))
    mb1c0 = np.tile((np.asarray(mb1, np.float32) + C[0])[None, :], (128, 1))
    mb2rep = np.tile(np.asarray(mb2, np.float32)[None, :], (128, 1))
    lngrep = np.tile(np.asarray(ln_g, np.float32)[None, :], (128, 1))
    lnbrep = np.tile(np.asarray(ln_b, np.float32)[None, :], (128, 1))
    cen = np.tile(np.linspace(0.0, CUTOFF, R, dtype=np.float32)[None, :],
                  (128, 1))
    iota = np.tile(np.arange(128, dtype=np.float32)[None, :], (128, 1))
    ident = np.eye(128, dtype=np.float32)
    uw1 = np.asarray(uw1, np.float32)

    shared = {
        "hT": hT,
        "w1a": W1a, "w1b": W1b, "wfeat": wfeat,
        "mw2": np.asarray(mw2, np.float32),
        "utop": np.ascontiguousarray(uw1[0:128]),
        "ubot": np.ascontiguousarray(uw1[128:256]),
        "uw2": np.asarray(uw2, np.float32),
        "ub1": np.asarray(ub1, np.float32).reshape(H, 1),
        "ub2": np.asarray(ub2, np.float32).reshape(H, 1),
        "mb1c0": np.ascontiguousarray(mb1c0),
        "mb2rep": np.ascontiguousarray(mb2rep),
        "lngrep": np.ascontiguousarray(lngrep),
        "lnbrep": np.ascontiguousarray(lnbrep),
        "centers": np.ascontiguousarray(cen),
        "iota": np.ascontiguousarray(iota),
        "ident": ident,
    }
    in_maps = []
    for c in range(NCORE):
        esrc2, edst2, edata2 = per_core[c]
        rows = slice(c * NB * 128, (c + 1) * NB * 128)
        m = dict(shared)
        m["h_own"] = np.ascontiguousarray(h_pad[rows])
        m["hTown"] = np.ascontiguousarray(hT[:, rows])
        m["maskf"] = np.ascontiguousarray(
            maskp[rows].reshape(NB, 128).T)
        m["esrc"] = esrc2
        m["edst"] = edst2
        m["edata"] = edata2
        in_maps.append(m)
    return tuple(int(x) for x in kc), in_maps


def kernel(**inputs):
    res = kernel_raw(**inputs)
    outs = [res.results[c]["out"] for c in range(NCORE)]
    full = np.concatenate(outs, axis=0)[:N]
    return np.ascontiguousarray(full.astype(np.float32))


def kernel_raw(_trace=False, **inputs):
    kc, in_maps = _prep(**inputs)
    if kc not in _cache:
        _cache[kc] = _build(kc)
    nc = _cache[kc]
    return run_bass_kernel_spmd(nc, in_maps, list(range(NCORE)), trace=_trace)



# revision 2
# speedup vs baseline: 1.0260x; 1.0260x over previous
"""GNN message-passing block v2: batched dma_gather + bf16 matmul pipeline.

Per core: 49 dst-node blocks, superblocks of 4 for gather batching.
Edge phase (edges on free axis, H on partitions):
  x1T[H,e] = silu(W1a@h[src]T + W1b@h[dst]T + wfeat@featT + mb1c0)  (psum acc)
  psy[e,H2] = ones@mb2 + x1T_chunk.T@mw2  (chunkwise matmul = fused transpose)
  msg = silu(psy);  sums[H,n] += msg.T @ onehot(dl)*w  (scatter-mean in PE)
Node phase: update MLP from sums, residual + LN stats; rsqrt deferred to one
batched Sqrt at the end (keeps the ACT table on the silu set throughout).
Ligand mask applied on host (exact h passthrough for non-ligand nodes).
"""

import sys

sys.path.insert(0, "/opt/trn_rl_repo")

import numpy as np
import ml_dtypes
from concourse import bacc, bass, mybir
from concourse.tile import TileContext
from concourse.bass_utils import run_bass_kernel_spmd

F32 = mybir.dt.float32
BF16 = mybir.dt.bfloat16
I16 = mybir.dt.int16
AF = mybir.ActivationFunctionType
ALU = mybir.AluOpType

N = 50000
E = 800000
H = 128
R = 32
CUTOFF = 6.0
NCORE = 8
NB = 49
NBLK = NCORE * NB
NPAD = NBLK * 128
GAMMA = 1.0 / max((CUTOFF / (R - 1)) ** 2, 1e-6)
LN_EPS = 1e-5
LO_LIMIT = 32768          # int16 idx cap for the lo gather view
HI_BASE = NPAD - 32768    # hi gather view base (17408)
SB_SIZES = [4] * 12 + [1]
GROUP = 4

BF = ml_dtypes.bfloat16
_cache = {}


def _sb_layout(kclo, kchi):
    """Column layout per superblock. Returns list of dicts."""
    sbs = []
    j0 = 0
    for sbn in SB_SIZES:
        blocks = list(range(j0, j0 + sbn))
        nlo = [kclo[j] for j in blocks]
        nhi = [kchi[j] for j in blocks]
        lo_off, off = [], 0
        for n_ in nlo:
            lo_off.append(off)
            off += n_ * 128
        wlo = off
        hi_off = []
        for n_ in nhi:
            hi_off.append(off)
            off += n_ * 128
        sbs.append(dict(j0=j0, blocks=blocks, nlo=nlo, nhi=nhi,
                        lo_off=lo_off, hi_off=hi_off, wlo=wlo,
                        whi=off - wlo, w=off, nch=off // 128))
        j0 += sbn
    return sbs


def _build(spec):
    kclo, kchi = spec
    sbs = _sb_layout(kclo, kchi)
    nch_tot = sum(sb["nch"] for sb in sbs)
    nch_max = max(sb["nch"] for sb in sbs)
    wmax = nch_max * 128

    nc = bacc.Bacc()
    h16 = nc.declare_dram_parameter("h16", [NPAD, H], BF16, isOutput=False)
    hTown = nc.declare_dram_parameter("hTown", [128, NB * 128], BF16,
                                      isOutput=False)
    h_own = nc.declare_dram_parameter("h_own", [NB * 128, H], F32,
                                      isOutput=False)
    idxall = nc.declare_dram_parameter("idxall", [128, 16 * nch_tot], I16,
                                       isOutput=False)
    featall = nc.declare_dram_parameter("featall", [34, 128 * nch_tot], BF16,
                                        isOutput=False)
    dlwall = nc.declare_dram_parameter("dlwall", [128, 2 * nch_tot], F32,
                                       isOutput=False)
    w1a = nc.declare_dram_parameter("w1a", [H, H], BF16, isOutput=False)
    w1b = nc.declare_dram_parameter("w1b", [H, H], BF16, isOutput=False)
    wfeat = nc.declare_dram_parameter("wfeat", [34, H], BF16, isOutput=False)
    mw2 = nc.declare_dram_parameter("mw2", [H, H], BF16, isOutput=False)
    utop = nc.declare_dram_parameter("utop", [H, H], BF16, isOutput=False)
    ubot = nc.declare_dram_parameter("ubot", [H, H], BF16, isOutput=False)
    uw2 = nc.declare_dram_parameter("uw2", [H, H], BF16, isOutput=False)
    ones1 = nc.declare_dram_parameter("ones1", [1, H], BF16, isOutput=False)
    mb2r4 = nc.declare_dram_parameter("mb2r4", [1, 512], BF16, isOutput=False)
    ub2row = nc.declare_dram_parameter("ub2row", [1, H], BF16, isOutput=False)
    mb1c0 = nc.declare_dram_parameter("mb1c0", [128, 1], F32, isOutput=False)
    ub1c = nc.declare_dram_parameter("ub1c", [128, 1], F32, isOutput=False)
    iota = nc.declare_dram_parameter("iota", [128, 128], F32, isOutput=False)
    epsc = nc.declare_dram_parameter("epsc", [128, 1], F32, isOutput=False)
    lngr = nc.declare_dram_parameter("lngr", [128, 128], F32, isOutput=False)
    lnbr = nc.declare_dram_parameter("lnbr", [128, 128], F32, isOutput=False)
    out = nc.declare_dram_parameter("out", [NB * 128, H], F32, isOutput=True)

    with TileContext(nc) as tc:
        with (
            tc.tile_pool(name="pc", bufs=1) as pc,
            tc.tile_pool(name="pidx", bufs=2) as pidx,
            tc.tile_pool(name="pft", bufs=2) as pft,
            tc.tile_pool(name="pdlw", bufs=2) as pdlw,
            tc.tile_pool(name="pg", bufs=2) as pg,
            tc.tile_pool(name="px1", bufs=3) as px1,
            tc.tile_pool(name="pms2", bufs=3) as pms2,
            tc.tile_pool(name="poh", bufs=4) as poh,
            tc.tile_pool(name="pnode", bufs=2) as pnode,
            tc.tile_pool(name="phb", bufs=2) as phb,
            tc.tile_pool(name="pfin", bufs=2) as pfin,
            tc.tile_pool(name="ppsx", bufs=2, space="PSUM") as ppsx,
            tc.tile_pool(name="ppsy", bufs=2, space="PSUM") as ppsy,
            tc.tile_pool(name="psums", bufs=2, space="PSUM") as psums,
            tc.tile_pool(name="ppsn", bufs=2, space="PSUM") as ppsn,
        ):
            def cload(ap, shape, tag, dtype=F32):
                t = pc.tile(shape, dtype, tag=tag)
                nc.sync.dma_start(out=t[:], in_=ap[:])
                return t

            w1a_t = cload(w1a, [H, H], "w1a", BF16)
            w1b_t = cload(w1b, [H, H], "w1b", BF16)
            wfeat_t = cload(wfeat, [34, H], "wfeat", BF16)
            mw2_t = cload(mw2, [H, H], "mw2", BF16)
            utop_t = cload(utop, [H, H], "utop", BF16)
            ubot_t = cload(ubot, [H, H], "ubot", BF16)
            uw2_t = cload(uw2, [H, H], "uw2", BF16)
            ones1_t = cload(ones1, [1, H], "ones1", BF16)
            mb2r4_t = cload(mb2r4, [1, 512], "mb2r4", BF16)
            ub2row_t = cload(ub2row, [1, H], "ub2row", BF16)
            mb1c0_t = cload(mb1c0, [128, 1], "mb1c0")
            ub1c_t = cload(ub1c, [128, 1], "ub1c")
            iota_t = cload(iota, [128, 128], "iota")
            epsc_t = cload(epsc, [128, 1], "epsc")
            lngr_t = cload(lngr, [128, 128], "lngr")
            lnbr_t = cload(lnbr, [128, 128], "lnbr")
            hTown_t = cload(hTown, [128, NB * 128], "hTown", BF16)

            zc_all = pc.tile([128, NB * 128], F32, tag="zc_all")
            ss_all = pc.tile([128, NB], F32, tag="ss_all")

            ibase = fbase = dbase = 0
            for sb in sbs:
                nch, w, wlo, whi = sb["nch"], sb["w"], sb["wlo"], sb["whi"]
                it = pidx.tile([128, 16 * nch_max], I16, tag="idx")
                nc.sync.dma_start(out=it[:, 0:16 * nch],
                                  in_=idxall[:, ibase:ibase + 16 * nch])
                ft = pft.tile([34, wmax], BF16, tag="ft")
                nc.sync.dma_start(out=ft[:, 0:w],
                                  in_=featall[:, fbase:fbase + w])
                dlw_t = pdlw.tile([128, 2 * nch_max], F32, tag="dlw")
                nc.sync.dma_start(out=dlw_t[:, 0:2 * nch],
                                  in_=dlwall[:, dbase:dbase + 2 * nch])

                def gather(dst_t, col0, view, icol0, n_):
                    for off in range(0, n_, 2048):
                        m = min(2048, n_ - off)
                        nc.gpsimd.dma_gather(
                            dst_t[:, :, col0 + off:col0 + off + m], view,
                            it[:, icol0 + off // 16:icol0 + (off + m) // 16],
                            m, m, H, transpose=True, single_packet=False)

                hsrc = pg.tile([128, 1, wmax], BF16, tag="hsrc")
                gather(hsrc, 0, h16[0:NPAD, :], 0, wlo)
                if whi:
                    gather(hsrc, wlo, h16[HI_BASE:NPAD, :], wlo // 16, whi)
                hdst = pg.tile([128, 1, wmax], BF16, tag="hdst")
                base = sb["j0"] * 128
                gather(hdst, 0, h16[base:min(base + 32768, NPAD), :],
                       8 * nch, w)

                for bi, j in enumerate(sb["blocks"]):
                    sums = psums.tile([128, 128], F32, tag="sums")
                    segs = [(sb["lo_off"][bi], sb["nlo"][bi]),
                            (sb["hi_off"][bi], sb["nhi"][bi])]
                    tot_ch = sb["nlo"][bi] + sb["nhi"][bi]
                    done_ch = 0
                    for seg_off, seg_n in segs:
                        for g0 in range(0, seg_n, GROUP):
                            gn = min(GROUP, seg_n - g0)
                            gw = gn * 128
                            c0 = seg_off + g0 * 128
                            cols = slice(c0, c0 + gw)
                            psx = ppsx.tile([128, 512], F32, tag="psx")
                            nc.tensor.matmul(psx[:, 0:gw], w1a_t[:],
                                             hsrc[:, 0, cols],
                                             start=True, stop=False)
                            nc.tensor.matmul(psx[:, 0:gw], w1b_t[:],
                                             hdst[:, 0, cols],
                                             start=False, stop=False)
                            nc.tensor.matmul(psx[:, 0:gw], wfeat_t[:],
                                             ft[:, cols],
                                             start=False, stop=True)
                            x1 = px1.tile([128, 512], BF16, tag="x1")
                            nc.scalar.activation(x1[:, 0:gw], psx[:, 0:gw],
                                                 AF.Silu,
                                                 bias=mb1c0_t[:, 0:1])
                            psy = ppsy.tile([128, 512], F32, tag="psy")
                            nc.tensor.matmul(psy[:, 0:gw], ones1_t[:],
                                             mb2r4_t[:, 0:gw],
                                             start=True, stop=False,
                                             skip_group_check=True)
                            for c in range(gn):
                                cc = slice(c * 128, (c + 1) * 128)
                                nc.tensor.matmul(psy[:, cc], x1[:, cc],
                                                 mw2_t[:],
                                                 start=False, stop=True,
                                                 skip_group_check=True)
                            ms2 = pms2.tile([128, 512], BF16, tag="ms2")
                            nc.scalar.activation(ms2[:, 0:gw], psy[:, 0:gw],
                                                 AF.Silu)
                            for c in range(gn):
                                ci = (c0 + c * 128) // 128
                                oh = poh.tile([128, 128], BF16, tag="oh")
                                nc.vector.tensor_scalar(
                                    oh[:], iota_t[:],
                                    dlw_t[:, 2 * ci:2 * ci + 1],
                                    dlw_t[:, 2 * ci + 1:2 * ci + 2],
                                    ALU.is_equal, ALU.mult)
                                nc.tensor.matmul(
                                    sums[:], ms2[:, c * 128:(c + 1) * 128],
                                    oh[:], start=(done_ch == 0),
                                    stop=(done_ch == tot_ch - 1),
                                    skip_group_check=True)
                                done_ch += 1

                    # node phase A
                    aggT = pnode.tile([128, 128], BF16, tag="aggT")
                    nc.vector.tensor_copy(aggT[:], sums[:])
                    psu = ppsn.tile([128, 128], F32, tag="psn")
                    nc.tensor.matmul(psu[:], utop_t[:],
                                     hTown_t[:, j * 128:(j + 1) * 128],
                                     start=True, stop=False)
                    nc.tensor.matmul(psu[:], ubot_t[:], aggT[:],
                                     start=False, stop=True)
                    us = pnode.tile([128, 128], BF16, tag="us")
                    nc.scalar.activation(us[:], psu[:], AF.Silu,
                                         bias=ub1c_t[:, 0:1])
                    psupd = ppsn.tile([128, 128], F32, tag="psn")
                    nc.tensor.matmul(psupd[:], ones1_t[:], ub2row_t[:],
                                     start=True, stop=False,
                                     skip_group_check=True)
                    nc.tensor.matmul(psupd[:], us[:], uw2_t[:],
                                     start=False, stop=True,
                                     skip_group_check=True)
                    hb = phb.tile([128, 128], F32, tag="hb")
                    nc.sync.dma_start(out=hb[:],
                                      in_=h_own[j * 128:(j + 1) * 128, :])
                    z = pnode.tile([128, 128], F32, tag="z")
                    nc.vector.tensor_tensor(z[:], psupd[:], hb[:], op=ALU.add)
                    mus = pnode.tile([128, 1], F32, tag="mus")
                    nc.vector.tensor_reduce(mus[:], z[:],
                                            mybir.AxisListType.X, ALU.add)
                    mu = pnode.tile([128, 1], F32, tag="mu")
                    nc.vector.tensor_scalar(mu[:], mus[:], 1.0 / H, None,
                                            ALU.mult)
                    zc = zc_all[:, j * 128:(j + 1) * 128]
                    nc.vector.tensor_scalar(zc, z[:], mu[:, 0:1], None,
                                            ALU.subtract)
                    sqs = pnode.tile([128, 128], BF16, tag="sqs")
                    nc.scalar.activation(sqs[:], zc, AF.Square,
                                         accum_out=ss_all[:, j:j + 1])
                ibase += 16 * nch
                fbase += 128 * nch
                dbase += 2 * nch

            # deferred LN tail: one Sqrt table load for the whole kernel
            sd = pfin.tile([128, NB], F32, tag="sd")
            nc.scalar.activation(sd[:], ss_all[:], AF.Sqrt,
                                 scale=1.0 / H, bias=epsc_t[:, 0:1])
            rstd = pfin.tile([128, NB], F32, tag="rstd")
            nc.vector.reciprocal(rstd[:], sd[:])
            for j in range(NB):
                nm = pfin.tile([128, 128], F32, tag="nm")
                nc.vector.tensor_scalar(nm[:], zc_all[:, j * 128:(j + 1) * 128],
                                        rstd[:, j:j + 1], None, ALU.mult)
                nc.vector.tensor_tensor(nm[:], nm[:], lngr_t[:], op=ALU.mult)
                nc.vector.tensor_tensor(nm[:], nm[:], lnbr_t[:], op=ALU.add)
                nc.sync.dma_start(out=out[j * 128:(j + 1) * 128, :], in_=nm[:])

    nc.compile()
    return nc


def _prep(h, pos, edge_index, edge_type, node_type,
          emb, mw1, mb1, mw2, mb2, uw1, ub1, uw2, ub2, ln_g, ln_b):
    h = np.asarray(h, np.float32)
    pos = np.asarray(pos, np.float32)
    src = np.asarray(edge_index[0], np.int64)
    dst = np.asarray(edge_index[1], np.int64)
    et = np.asarray(edge_type, np.int64).astype(np.float32)
    mw1 = np.asarray(mw1, np.float32)
    emb = np.asarray(emb, np.float32)

    cnt = np.bincount(dst, minlength=N).astype(np.float32)
    w_all = 1.0 / np.maximum(cnt, 1.0)
    rel = pos[src] - pos[dst]
    dist = np.sqrt((rel * rel).sum(axis=1)).astype(np.float32)

    blk = dst >> 7
    order0 = np.argsort(blk, kind="stable")
    bc = np.bincount(blk, minlength=NBLK)
    bstart = np.zeros(NBLK + 1, np.int64)
    np.cumsum(bc, out=bstart[1:])

    # per (core, block): lo/hi edge id lists with midpoint balancing
    lo_lists = [[None] * NB for _ in range(NCORE)]
    hi_lists = [[None] * NB for _ in range(NCORE)]
    kclo = np.zeros(NB, np.int64)
    kchi = np.zeros(NB, np.int64)
    for c in range(NCORE):
        shift = c * NB * 128
        for j in range(NB):
            g = c * NB + j
            ids = order0[bstart[g]:bstart[g + 1]]
            s = (src[ids] - shift) % NPAD
            ml = ids[s < HI_BASE]
            mh = ids[s >= LO_LIMIT]
            mm = ids[(s >= HI_BASE) & (s < LO_LIMIT)]
            x = min(len(mm), (-len(ml)) % 128)
            lo = np.concatenate([ml, mm[:x]])
            hi = np.concatenate([mm[x:], mh])
            lo_lists[c][j] = lo
            hi_lists[c][j] = hi
            kclo[j] = max(kclo[j], (len(lo) + 127) // 128, 1)
            kchi[j] = max(kchi[j], (len(hi) + 127) // 128)

    spec = (tuple(int(x) for x in kclo), tuple(int(x) for x in kchi))
    sbs = _sb_layout(*spec)
    nch_tot = sum(sb["nch"] for sb in sbs)
    wtot = 128 * nch_tot

    centers = np.linspace(0.0, CUTOFF, R, dtype=np.float32)

    in_maps = []
    hpad = np.zeros((NPAD, H), np.float32)
    hpad[:N] = h
    h16 = hpad.astype(BF)

    W1a = np.ascontiguousarray(mw1[0:128]).astype(BF)
    W1b = np.ascontiguousarray(mw1[128:256]).astype(BF)
    C = emb @ mw1[256:384]
    wfeat = np.vstack([mw1[384:416], mw1[416:417],
                       (C[1] - C[0])[None, :]]).astype(BF)
    mb2v = np.asarray(mb2, np.float32)
    shared = {
        "h16": h16,
        "w1a": W1a, "w1b": W1b, "wfeat": np.ascontiguousarray(wfeat),
        "mw2": np.asarray(mw2, np.float32).astype(BF),
        "utop": np.ascontiguousarray(np.asarray(uw1, np.float32)[0:128]).astype(BF),
        "ubot": np.ascontiguousarray(np.asarray(uw1, np.float32)[128:256]).astype(BF),
        "uw2": np.asarray(uw2, np.float32).astype(BF),
        "ones1": np.ones((1, H), BF),
        "mb2r4": np.ascontiguousarray(np.tile(mb2v, 4)[None, :]).astype(BF),
        "ub2row": np.asarray(ub2, np.float32)[None, :].astype(BF),
        "mb1c0": (np.asarray(mb1, np.float32) + C[0]).reshape(H, 1).astype(np.float32),
        "ub1c": np.asarray(ub1, np.float32).reshape(H, 1),
        "iota": np.tile(np.arange(128, dtype=np.float32)[None, :], (128, 1)),
        "epsc": np.full((128, 1), LN_EPS, np.float32),
        "lngr": np.tile(np.asarray(ln_g, np.float32)[None, :], (128, 1)),
        "lnbr": np.tile(np.asarray(ln_b, np.float32)[None, :], (128, 1)),
    }

    for c in range(NCORE):
        shift = c * NB * 128
        sel = np.full(wtot, -1, np.int64)
        pos_ = 0
        for sb in sbs:
            for bi, j in enumerate(sb["blocks"]):
                lo = lo_lists[c][j]
                sel[pos_ + sb["lo_off"][bi]:
                    pos_ + sb["lo_off"][bi] + len(lo)] = lo
            for bi, j in enumerate(sb["blocks"]):
                hi = hi_lists[c][j]
                sel[pos_ + sb["hi_off"][bi]:
                    pos_ + sb["hi_off"][bi] + len(hi)] = hi
            pos_ += sb["w"]

        valid = sel >= 0
        selc = np.where(valid, sel, 0)
        s_ = (src[selc] - shift) % NPAD   # rotated node space
        d_rot = (dst[selc] - shift) % NPAD
        d_ = dst[selc]                     # global (for counts / dl bits)

        # featT [34, wtot]
        dist_e = np.where(valid, dist[selc], 0.0).astype(np.float32)
        rad = np.exp(-GAMMA * (dist_e[:, None] - centers[None, :]) ** 2)
        rad[~valid] = 0.0
        feat = np.zeros((34, wtot), np.float32)
        feat[0:32] = rad.T
        feat[32] = dist_e
        feat[33] = np.where(valid, et[selc], 0.0)

        # dlw [128, 2*nch]
        dl = np.where(valid, (d_ & 127).astype(np.float32), -1.0)
        wv = np.where(valid, w_all[d_], 0.0).astype(np.float32)
        dlw = np.zeros((128, 2 * nch_tot), np.float32)
        dlw[:, 0::2] = dl.reshape(nch_tot, 128).T
        dlw[:, 1::2] = wv.reshape(nch_tot, 128).T

        # idx sections per sb
        idxall = np.zeros((128, 16 * nch_tot), np.int16)
        pos_ = 0
        icol = 0
        for sb in sbs:
            wsb, wlo = sb["w"], sb["wlo"]
            ssb = s_[pos_:pos_ + wsb].copy()
            vsb = valid[pos_:pos_ + wsb]
            srel = np.where(vsb, ssb, 0)
            srel[wlo:] = np.where(vsb[wlo:], ssb[wlo:] - HI_BASE, 0)
            drel = np.where(vsb, d_rot[pos_:pos_ + wsb] - sb["j0"] * 128, 0)
            sec = np.concatenate([srel, drel]).astype(np.int16)
            blk16 = sec.reshape(2 * wsb // 16, 16).T
            idxall[:, icol:icol + 2 * wsb // 16] = np.tile(blk16, (8, 1))
            pos_ += wsb
            icol += 2 * wsb // 16

        rows = slice(c * NB * 128, (c + 1) * NB * 128)
        m = dict(shared)
        m["h16"] = np.roll(h16, -shift, axis=0)
        m["hTown"] = np.ascontiguousarray(h16[rows].T)
        m["h_own"] = np.ascontiguousarray(hpad[rows])
        m["idxall"] = idxall
        m["featall"] = np.ascontiguousarray(feat).astype(BF)
        m["dlwall"] = dlw
        in_maps.append(m)
    return spec, in_maps


def kernel(**inputs):
    res = kernel_raw(**inputs)
    outs = [res.results[c]["out"] for c in range(NCORE)]
    full = np.concatenate(outs, axis=0)[:N]
    h = np.asarray(inputs["h"], np.float32)
    mask = (np.asarray(inputs["node_type"]) == 0)[:, None]
    return np.ascontiguousarray(
        np.where(mask, full.astype(np.float32), h))


def kernel_raw(_trace=False, **inputs):
    spec, in_maps = _prep(**inputs)
    if spec not in _cache:
        _cache[spec] = _build(spec)
    return run_bass_kernel_spmd(_cache[spec], in_maps,
                                list(range(NCORE)), trace=_trace)


# revision 3
# speedup vs baseline: 1.5804x; 1.5404x over previous
"""GNN message-passing block v2: batched dma_gather + bf16 matmul pipeline.

Per core: 49 dst-node blocks, superblocks of 4 for gather batching.
Edge phase (edges on free axis, H on partitions):
  x1T[H,e] = silu(W1a@h[src]T + W1b@h[dst]T + wfeat@featT + mb1c0)  (psum acc)
  psy[e,H2] = ones@mb2 + x1T_chunk.T@mw2  (chunkwise matmul = fused transpose)
  msg = silu(psy);  sums[H,n] += msg.T @ onehot(dl)*w  (scatter-mean in PE)
Node phase: update MLP from sums, residual + LN stats; rsqrt deferred to one
batched Sqrt at the end (keeps the ACT table on the silu set throughout).
Ligand mask applied on host (exact h passthrough for non-ligand nodes).
"""

import sys

sys.path.insert(0, "/opt/trn_rl_repo")

import numpy as np
import ml_dtypes
from concourse import bacc, bass, mybir
from concourse.tile import TileContext
from concourse.bass_utils import run_bass_kernel_spmd

F32 = mybir.dt.float32
BF16 = mybir.dt.bfloat16
I16 = mybir.dt.int16
AF = mybir.ActivationFunctionType
ALU = mybir.AluOpType

N = 50000
E = 800000
H = 128
R = 32
CUTOFF = 6.0
NCORE = 8
NB = 49
NBLK = NCORE * NB
NPAD = NBLK * 128
GAMMA = 1.0 / max((CUTOFF / (R - 1)) ** 2, 1e-6)
LN_EPS = 1e-5
LO_LIMIT = 32768          # int16 idx cap for the lo gather view
HI_BASE = NPAD - 32768    # hi gather view base (17408)
SB_SIZES = [4] * 12 + [1]
GROUP = 4

BF = ml_dtypes.bfloat16
_cache = {}


def _sb_layout(kclo, kchi):
    """Column layout per superblock. Returns list of dicts."""
    sbs = []
    j0 = 0
    for sbn in SB_SIZES:
        blocks = list(range(j0, j0 + sbn))
        nlo = [kclo[j] for j in blocks]
        nhi = [kchi[j] for j in blocks]
        lo_off, off = [], 0
        for n_ in nlo:
            lo_off.append(off)
            off += n_ * 128
        wlo = off
        hi_off = []
        for n_ in nhi:
            hi_off.append(off)
            off += n_ * 128
        sbs.append(dict(j0=j0, blocks=blocks, nlo=nlo, nhi=nhi,
                        lo_off=lo_off, hi_off=hi_off, wlo=wlo,
                        whi=off - wlo, w=off, nch=off // 128))
        j0 += sbn
    return sbs


def _build(spec):
    kclo, kchi = spec
    sbs = _sb_layout(kclo, kchi)
    nch_tot = sum(sb["nch"] for sb in sbs)
    nch_max = max(sb["nch"] for sb in sbs)
    wmax = nch_max * 128

    nc = bacc.Bacc()
    h16 = nc.declare_dram_parameter("h16", [NPAD, H], BF16, isOutput=False)
    hTown = nc.declare_dram_parameter("hTown", [128, NB * 128], BF16,
                                      isOutput=False)
    h_own = nc.declare_dram_parameter("h_own", [NB * 128, H], F32,
                                      isOutput=False)
    idxall = nc.declare_dram_parameter("idxall", [128, 16 * nch_tot], I16,
                                       isOutput=False)
    featall = nc.declare_dram_parameter("featall", [34, 128 * nch_tot], BF16,
                                        isOutput=False)
    ohwall = nc.declare_dram_parameter("ohwall", [128, 128 * nch_tot], BF16,
                                       isOutput=False)
    w1a = nc.declare_dram_parameter("w1a", [H, H], BF16, isOutput=False)
    w1b = nc.declare_dram_parameter("w1b", [H, H], BF16, isOutput=False)
    wfeat = nc.declare_dram_parameter("wfeat", [34, H], BF16, isOutput=False)
    mw2 = nc.declare_dram_parameter("mw2", [H, H], BF16, isOutput=False)
    utop = nc.declare_dram_parameter("utop", [H, H], BF16, isOutput=False)
    ubot = nc.declare_dram_parameter("ubot", [H, H], BF16, isOutput=False)
    uw2 = nc.declare_dram_parameter("uw2", [H, H], BF16, isOutput=False)
    ones1 = nc.declare_dram_parameter("ones1", [1, H], BF16, isOutput=False)
    mb2r4 = nc.declare_dram_parameter("mb2r4", [1, 512], BF16, isOutput=False)
    ub2row = nc.declare_dram_parameter("ub2row", [1, H], BF16, isOutput=False)
    mb1c0 = nc.declare_dram_parameter("mb1c0", [128, 1], F32, isOutput=False)
    ub1c = nc.declare_dram_parameter("ub1c", [128, 1], F32, isOutput=False)
    iota = nc.declare_dram_parameter("iota", [128, 128], F32, isOutput=False)
    epsc = nc.declare_dram_parameter("epsc", [128, 1], F32, isOutput=False)
    lngr = nc.declare_dram_parameter("lngr", [128, 128], F32, isOutput=False)
    lnbr = nc.declare_dram_parameter("lnbr", [128, 128], F32, isOutput=False)
    out = nc.declare_dram_parameter("out", [NB * 128, H], F32, isOutput=True)

    with TileContext(nc) as tc:
        with (
            tc.tile_pool(name="pc", bufs=1) as pc,
            tc.tile_pool(name="pidx", bufs=2) as pidx,
            tc.tile_pool(name="pft", bufs=2) as pft,
            tc.tile_pool(name="pdlw", bufs=2) as pdlw,
            tc.tile_pool(name="pg", bufs=2) as pg,
            tc.tile_pool(name="px1", bufs=2) as px1,
            tc.tile_pool(name="pms2", bufs=2) as pms2,
            tc.tile_pool(name="poh", bufs=4) as poh,
            tc.tile_pool(name="pnode", bufs=2) as pnode,
            tc.tile_pool(name="phb", bufs=2) as phb,
            tc.tile_pool(name="pfin", bufs=2) as pfin,
            tc.tile_pool(name="ppsx", bufs=2, space="PSUM") as ppsx,
            tc.tile_pool(name="ppsy", bufs=2, space="PSUM") as ppsy,
            tc.tile_pool(name="psums", bufs=2, space="PSUM") as psums,
            tc.tile_pool(name="ppsn", bufs=2, space="PSUM") as ppsn,
        ):
            def cload(ap, shape, tag, dtype=F32):
                t = pc.tile(shape, dtype, tag=tag)
                nc.sync.dma_start(out=t[:], in_=ap[:])
                return t

            w1a_t = cload(w1a, [H, H], "w1a", BF16)
            w1b_t = cload(w1b, [H, H], "w1b", BF16)
            wfeat_t = cload(wfeat, [34, H], "wfeat", BF16)
            mw2_t = cload(mw2, [H, H], "mw2", BF16)
            utop_t = cload(utop, [H, H], "utop", BF16)
            ubot_t = cload(ubot, [H, H], "ubot", BF16)
            uw2_t = cload(uw2, [H, H], "uw2", BF16)
            ones1_t = cload(ones1, [1, H], "ones1", BF16)
            mb2r4_t = cload(mb2r4, [1, 512], "mb2r4", BF16)
            ub2row_t = cload(ub2row, [1, H], "ub2row", BF16)
            mb1c0_t = cload(mb1c0, [128, 1], "mb1c0")
            ub1c_t = cload(ub1c, [128, 1], "ub1c")
            epsc_t = cload(epsc, [128, 1], "epsc")
            lngr_t = cload(lngr, [128, 128], "lngr")
            lnbr_t = cload(lnbr, [128, 128], "lnbr")
            hTown_t = cload(hTown, [128, NB * 128], "hTown", BF16)

            zc_all = pc.tile([128, NB * 128], F32, tag="zc_all")
            ss_all = pc.tile([128, NB], F32, tag="ss_all")

            ibase = fbase = dbase = 0
            for sb in sbs:
                nch, w, wlo, whi = sb["nch"], sb["w"], sb["wlo"], sb["whi"]
                it = pidx.tile([128, 16 * nch_max], I16, tag="idx")
                nc.sync.dma_start(out=it[:, 0:16 * nch],
                                  in_=idxall[:, ibase:ibase + 16 * nch])
                ft = pft.tile([34, wmax], BF16, tag="ft")
                nc.sync.dma_start(out=ft[:, 0:w],
                                  in_=featall[:, fbase:fbase + w])
                ohl = pdlw.tile([128, wmax], BF16, tag="ohl")
                nc.sync.dma_start(out=ohl[:, 0:w],
                                  in_=ohwall[:, fbase:fbase + w])

                def gather(dst_t, col0, view, icol0, n_):
                    for off in range(0, n_, 2048):
                        m = min(2048, n_ - off)
                        nc.gpsimd.dma_gather(
                            dst_t[:, :, col0 + off:col0 + off + m], view,
                            it[:, icol0 + off // 16:icol0 + (off + m) // 16],
                            m, m, H, transpose=True, single_packet=False)

                hsrc = pg.tile([128, 1, wmax], BF16, tag="hsrc")
                gather(hsrc, 0, h16[0:NPAD, :], 0, wlo)
                if whi:
                    gather(hsrc, wlo, h16[HI_BASE:NPAD, :], wlo // 16, whi)
                hdst = pg.tile([128, 1, wmax], BF16, tag="hdst")
                base = sb["j0"] * 128
                gather(hdst, 0, h16[base:min(base + 32768, NPAD), :],
                       8 * nch, w)

                for bi, j in enumerate(sb["blocks"]):
                    sums = psums.tile([128, 128], F32, tag="sums")
                    segs = [(sb["lo_off"][bi], sb["nlo"][bi]),
                            (sb["hi_off"][bi], sb["nhi"][bi])]
                    tot_ch = sb["nlo"][bi] + sb["nhi"][bi]
                    done_ch = 0
                    for seg_off, seg_n in segs:
                        for g0 in range(0, seg_n, GROUP):
                            gn = min(GROUP, seg_n - g0)
                            gw = gn * 128
                            c0 = seg_off + g0 * 128
                            cols = slice(c0, c0 + gw)
                            psx = ppsx.tile([128, 512], F32, tag="psx")
                            nc.tensor.matmul(psx[:, 0:gw], w1a_t[:],
                                             hsrc[:, 0, cols],
                                             start=True, stop=False)
                            nc.tensor.matmul(psx[:, 0:gw], w1b_t[:],
                                             hdst[:, 0, cols],
                                             start=False, stop=False)
                            nc.tensor.matmul(psx[:, 0:gw], wfeat_t[:],
                                             ft[:, cols],
                                             start=False, stop=True)
                            x1 = px1.tile([128, 512], BF16, tag="x1")
                            nc.scalar.activation(x1[:, 0:gw], psx[:, 0:gw],
                                                 AF.Silu,
                                                 bias=mb1c0_t[:, 0:1])
                            psy = ppsy.tile([128, 512], F32, tag="psy")
                            nc.tensor.matmul(psy[:, 0:gw], ones1_t[:],
                                             mb2r4_t[:, 0:gw],
                                             start=True, stop=False,
                                             skip_group_check=True)
                            for c in range(gn):
                                cc = slice(c * 128, (c + 1) * 128)
                                nc.tensor.matmul(psy[:, cc], x1[:, cc],
                                                 mw2_t[:],
                                                 start=False, stop=True,
                                                 skip_group_check=True)
                            ms2 = pms2.tile([128, 512], BF16, tag="ms2")
                            nc.scalar.activation(ms2[:, 0:gw], psy[:, 0:gw],
                                                 AF.Silu)
                            for c in range(gn):
                                cc = slice(c0 + c * 128, c0 + (c + 1) * 128)
                                nc.tensor.matmul(
                                    sums[:], ms2[:, c * 128:(c + 1) * 128],
                                    ohl[:, cc], start=(done_ch == 0),
                                    stop=(done_ch == tot_ch - 1),
                                    skip_group_check=True)
                                done_ch += 1

                    # node phase A
                    aggT = pnode.tile([128, 128], BF16, tag="aggT")
                    nc.vector.tensor_copy(aggT[:], sums[:])
                    psu = ppsn.tile([128, 128], F32, tag="psn")
                    nc.tensor.matmul(psu[:], utop_t[:],
                                     hTown_t[:, j * 128:(j + 1) * 128],
                                     start=True, stop=False)
                    nc.tensor.matmul(psu[:], ubot_t[:], aggT[:],
                                     start=False, stop=True)
                    us = pnode.tile([128, 128], BF16, tag="us")
                    nc.scalar.activation(us[:], psu[:], AF.Silu,
                                         bias=ub1c_t[:, 0:1])
                    psupd = ppsn.tile([128, 128], F32, tag="psn")
                    nc.tensor.matmul(psupd[:], ones1_t[:], ub2row_t[:],
                                     start=True, stop=False,
                                     skip_group_check=True)
                    nc.tensor.matmul(psupd[:], us[:], uw2_t[:],
                                     start=False, stop=True,
                                     skip_group_check=True)
                    hb = phb.tile([128, 128], F32, tag="hb")
                    nc.sync.dma_start(out=hb[:],
                                      in_=h_own[j * 128:(j + 1) * 128, :])
                    z = pnode.tile([128, 128], F32, tag="z")
                    nc.vector.tensor_tensor(z[:], psupd[:], hb[:], op=ALU.add)
                    mus = pnode.tile([128, 1], F32, tag="mus")
                    nc.vector.tensor_reduce(mus[:], z[:],
                                            mybir.AxisListType.X, ALU.add)
                    mu = pnode.tile([128, 1], F32, tag="mu")
                    nc.vector.tensor_scalar(mu[:], mus[:], 1.0 / H, None,
                                            ALU.mult)
                    zc = zc_all[:, j * 128:(j + 1) * 128]
                    nc.vector.tensor_scalar(zc, z[:], mu[:, 0:1], None,
                                            ALU.subtract)
                    sqs = pnode.tile([128, 128], BF16, tag="sqs")
                    nc.scalar.activation(sqs[:], zc, AF.Square,
                                         accum_out=ss_all[:, j:j + 1])
                ibase += 16 * nch
                fbase += 128 * nch
                dbase += 2 * nch

            # deferred LN tail: one Sqrt table load for the whole kernel
            sd = pfin.tile([128, NB], F32, tag="sd")
            nc.scalar.activation(sd[:], ss_all[:], AF.Sqrt,
                                 scale=1.0 / H, bias=epsc_t[:, 0:1])
            rstd = pfin.tile([128, NB], F32, tag="rstd")
            nc.vector.reciprocal(rstd[:], sd[:])
            for j in range(NB):
                nm = pfin.tile([128, 128], F32, tag="nm")
                nc.vector.tensor_scalar(nm[:], zc_all[:, j * 128:(j + 1) * 128],
                                        rstd[:, j:j + 1], None, ALU.mult)
                nc.vector.tensor_tensor(nm[:], nm[:], lngr_t[:], op=ALU.mult)
                nc.vector.tensor_tensor(nm[:], nm[:], lnbr_t[:], op=ALU.add)
                nc.sync.dma_start(out=out[j * 128:(j + 1) * 128, :], in_=nm[:])

    nc.compile()
    return nc


def _prep(h, pos, edge_index, edge_type, node_type,
          emb, mw1, mb1, mw2, mb2, uw1, ub1, uw2, ub2, ln_g, ln_b):
    h = np.asarray(h, np.float32)
    pos = np.asarray(pos, np.float32)
    src = np.asarray(edge_index[0], np.int64)
    dst = np.asarray(edge_index[1], np.int64)
    et = np.asarray(edge_type, np.int64).astype(np.float32)
    mw1 = np.asarray(mw1, np.float32)
    emb = np.asarray(emb, np.float32)

    cnt = np.bincount(dst, minlength=N).astype(np.float32)
    w_all = 1.0 / np.maximum(cnt, 1.0)
    rel = pos[src] - pos[dst]
    dist = np.sqrt((rel * rel).sum(axis=1)).astype(np.float32)

    blk = dst >> 7
    order0 = np.argsort(blk, kind="stable")
    bc = np.bincount(blk, minlength=NBLK)
    bstart = np.zeros(NBLK + 1, np.int64)
    np.cumsum(bc, out=bstart[1:])

    # per (core, block): lo/hi edge id lists with midpoint balancing
    lo_lists = [[None] * NB for _ in range(NCORE)]
    hi_lists = [[None] * NB for _ in range(NCORE)]
    kclo = np.zeros(NB, np.int64)
    kchi = np.zeros(NB, np.int64)
    for c in range(NCORE):
        shift = c * NB * 128
        for j in range(NB):
            g = c * NB + j
            ids = order0[bstart[g]:bstart[g + 1]]
            s = (src[ids] - shift) % NPAD
            ml = ids[s < HI_BASE]
            mh = ids[s >= LO_LIMIT]
            mm = ids[(s >= HI_BASE) & (s < LO_LIMIT)]
            x = min(len(mm), (-len(ml)) % 128)
            lo = np.concatenate([ml, mm[:x]])
            hi = np.concatenate([mm[x:], mh])
            lo_lists[c][j] = lo
            hi_lists[c][j] = hi
            kclo[j] = max(kclo[j], (len(lo) + 127) // 128, 1)
            kchi[j] = max(kchi[j], (len(hi) + 127) // 128)

    spec = (tuple(int(x) for x in kclo), tuple(int(x) for x in kchi))
    sbs = _sb_layout(*spec)
    nch_tot = sum(sb["nch"] for sb in sbs)
    wtot = 128 * nch_tot

    centers = np.linspace(0.0, CUTOFF, R, dtype=np.float32)

    in_maps = []
    hpad = np.zeros((NPAD, H), np.float32)
    hpad[:N] = h
    h16 = hpad.astype(BF)

    W1a = np.ascontiguousarray(mw1[0:128]).astype(BF)
    W1b = np.ascontiguousarray(mw1[128:256]).astype(BF)
    C = emb @ mw1[256:384]
    wfeat = np.vstack([mw1[384:416], mw1[416:417],
                       (C[1] - C[0])[None, :]]).astype(BF)
    mb2v = np.asarray(mb2, np.float32)
    shared = {
        "h16": h16,
        "w1a": W1a, "w1b": W1b, "wfeat": np.ascontiguousarray(wfeat),
        "mw2": np.asarray(mw2, np.float32).astype(BF),
        "utop": np.ascontiguousarray(np.asarray(uw1, np.float32)[0:128]).astype(BF),
        "ubot": np.ascontiguousarray(np.asarray(uw1, np.float32)[128:256]).astype(BF),
        "uw2": np.asarray(uw2, np.float32).astype(BF),
        "ones1": np.ones((1, H), BF),
        "mb2r4": np.ascontiguousarray(np.tile(mb2v, 4)[None, :]).astype(BF),
        "ub2row": np.asarray(ub2, np.float32)[None, :].astype(BF),
        "mb1c0": (np.asarray(mb1, np.float32) + C[0]).reshape(H, 1).astype(np.float32),
        "ub1c": np.asarray(ub1, np.float32).reshape(H, 1),
        "iota": np.tile(np.arange(128, dtype=np.float32)[None, :], (128, 1)),
        "epsc": np.full((128, 1), LN_EPS, np.float32),
        "lngr": np.tile(np.asarray(ln_g, np.float32)[None, :], (128, 1)),
        "lnbr": np.tile(np.asarray(ln_b, np.float32)[None, :], (128, 1)),
    }

    for c in range(NCORE):
        shift = c * NB * 128
        sel = np.full(wtot, -1, np.int64)
        pos_ = 0
        for sb in sbs:
            for bi, j in enumerate(sb["blocks"]):
                lo = lo_lists[c][j]
                sel[pos_ + sb["lo_off"][bi]:
                    pos_ + sb["lo_off"][bi] + len(lo)] = lo
            for bi, j in enumerate(sb["blocks"]):
                hi = hi_lists[c][j]
                sel[pos_ + sb["hi_off"][bi]:
                    pos_ + sb["hi_off"][bi] + len(hi)] = hi
            pos_ += sb["w"]

        valid = sel >= 0
        selc = np.where(valid, sel, 0)
        s_ = (src[selc] - shift) % NPAD   # rotated node space
        d_rot = (dst[selc] - shift) % NPAD
        d_ = dst[selc]                     # global (for counts / dl bits)

        # featT [34, wtot]
        dist_e = np.where(valid, dist[selc], 0.0).astype(np.float32)
        rad = np.exp(-GAMMA * (dist_e[:, None] - centers[None, :]) ** 2)
        rad[~valid] = 0.0
        feat = np.zeros((34, wtot), np.float32)
        feat[0:32] = rad.T
        feat[32] = dist_e
        feat[33] = np.where(valid, et[selc], 0.0)

        # host-precomputed weighted one-hot scatter matrices [128e, nch*128n]
        dl = np.where(valid, (d_ & 127).astype(np.float32), -1.0)
        wv = np.where(valid, w_all[d_], 0.0).astype(np.float32)
        dl3 = dl.reshape(nch_tot, 128)
        w3 = wv.reshape(nch_tot, 128)
        oh3 = (dl3[:, :, None] ==
               np.arange(128, dtype=np.float32)[None, None, :])
        ohw3 = (oh3 * w3[:, :, None]).astype(BF)
        ohwall = np.ascontiguousarray(
            ohw3.transpose(1, 0, 2).reshape(128, nch_tot * 128))

        # idx sections per sb
        idxall = np.zeros((128, 16 * nch_tot), np.int16)
        pos_ = 0
        icol = 0
        for sb in sbs:
            wsb, wlo = sb["w"], sb["wlo"]
            ssb = s_[pos_:pos_ + wsb].copy()
            vsb = valid[pos_:pos_ + wsb]
            srel = np.where(vsb, ssb, 0)
            srel[wlo:] = np.where(vsb[wlo:], ssb[wlo:] - HI_BASE, 0)
            drel = np.where(vsb, d_rot[pos_:pos_ + wsb] - sb["j0"] * 128, 0)
            sec = np.concatenate([srel, drel]).astype(np.int16)
            blk16 = sec.reshape(2 * wsb // 16, 16).T
            idxall[:, icol:icol + 2 * wsb // 16] = np.tile(blk16, (8, 1))
            pos_ += wsb
            icol += 2 * wsb // 16

        rows = slice(c * NB * 128, (c + 1) * NB * 128)
        m = dict(shared)
        m["h16"] = np.roll(h16, -shift, axis=0)
        m["hTown"] = np.ascontiguousarray(h16[rows].T)
        m["h_own"] = np.ascontiguousarray(hpad[rows])
        m["idxall"] = idxall
        m["featall"] = np.ascontiguousarray(feat).astype(BF)
        m["ohwall"] = ohwall
        in_maps.append(m)
    return spec, in_maps


def kernel(**inputs):
    res = kernel_raw(**inputs)
    outs = [res.results[c]["out"] for c in range(NCORE)]
    full = np.concatenate(outs, axis=0)[:N]
    h = np.asarray(inputs["h"], np.float32)
    mask = (np.asarray(inputs["node_type"]) == 0)[:, None]
    return np.ascontiguousarray(
        np.where(mask, full.astype(np.float32), h))


def kernel_raw(_trace=False, **inputs):
    spec, in_maps = _prep(**inputs)
    if spec not in _cache:
        _cache[spec] = _build(spec)
    return run_bass_kernel_spmd(_cache[spec], in_maps,
                                list(range(NCORE)), trace=_trace)
